# revision 61
# baseline (speedup 1.0000x reference)
"""GQA attention kernel for Trainium2, sharded over 8 NeuronCores.

Problem: B=2, S=2048, D=2048, H=16 query heads, KV=4 kv heads, HD=128,
RoPE, no causal mask, out = softmax(q k^T / sqrt(HD)) v @ Wo.

Sharding: core = b*4 + g  (b in {0,1} batch, g in {0..3} head group).
Each core handles 4 query heads [4g..4g+3] and kv head g (exact GQA
split), with Wq/Wk/Wv column-sliced and Wo row-sliced.  Each core
produces a partial o_proj output for its batch; host sums the 4 partials
per batch.

Key layout/precision strategy (v2):
  - Q/K/V projections run as fp8e4m3 DoubleRow matmuls (2 K-chunks per
    instruction, 0.5 cyc/col): h and the weights are split on the host
    into hi + lo fp8 parts (power-of-2 pre-scales keep values in e4m3's
    normal range) and the product uses 3 terms hh*wh + hl*wh + hh*wl,
    which lands at bf16-level accuracy at ~0.75x the bf16 PE cost.
    De-scales fold into the (host-owned) cos/sin tables and the softmax
    denominator's ones-vector, so no extra on-chip ops.
  - RoPE applied in fp32 with head-broadcast APs; q/k transposed to
    qT/kT [HD, S] via PE; last attention block's q transposes deferred
    past the phase boundary so kT completes as early as possible.
  - scoresT tiles = matmul(lhsT=kT_chunk, rhs=qT_block) in bf16, PAIRS
    into one 2-bank [128,1024] PSUM tile so a single wide exp on
    ScalarE covers both; exp straight out of PSUM, no max subtraction
    (scores ~N(0,1), |s| < ~6.5, safely inside fp32/exp range).
  - PV is FLIPPED: out_q[q, hd] = matmul(lhsT=expT_slice[keys, q_sub],
    rhs=v_chunk[keys, hd]).  The exp tile is the stationary operand, so
    the softmax denominators ride along as extra matmuls with a
    [128,1]-column ones vector (ap size 1 -> ~free on the PE: same
    stationary operand, one extra streamed column).  This removes the
    old dedicated denominator matmul stream (~54us of PE time).
    All four q-subchunk groups are emitted per score pair (PE work per
    pair ~1.28us > ScalarE exp ~1.04us, so the exp stream latency is
    self-covered); the groups accumulate with start=False onto a
    freshly zeroed bank (a zeros-weights matmul, kept on the PE), since
    per-instruction start bits cannot express four interleaved column
    groups in one 2KB zero region.
  - normalization: per-q reciprocal of the denominators lands on q
    partitions, exactly matching the flipped PV output, so it is a
    single [128,4,1]-broadcast multiply; the normalized tile is then
    PE-transposed back to aoT [c, q] for o_proj.
  - o_proj: y_piece = sum_ch matmul(lhsT=aoT chunk, rhs=Wo chunk) at
    [128,512]; pieces for block b-1 are emitted inside block b's head
    loop so the PE fills any exp-latency gaps with o_proj work.
  - cross-head software pipelining: every head pre-emits the NEXT
    head's 8 score pairs (throttled by the 20-deep exp pool), so
    ScalarE streams exps continuously across head/block boundaries;
    block-0 head-0's score pairs are emitted inside phase 1 (ps_q is
    banished to the "y" PSUM slots there, keeping the wide slots free),
    so attention starts with its exps already resident.
  - DMA ladder ordered by consumption time (wq_hi+ht0 first, lo parts
    behind, cos/sin trailing into the loop); hT tiles prefetched 3-4
    ahead through a 6-buf pool.
  - PSUM (8 banks): wide 2x2 (score pairs / ph1 ps_q tiles 0-1 /
    o_proj tail), a 1x1 (ph1 transposes / ps_oq + anorm transposes),
    c 1x1 (ph1 ps_kv / denominators), y 2x1 (ph1 ps_q / o_proj pieces).

TimelineSim: 258.7us/core (PE busy ~233.4us, 90% occupancy) vs the
329.6us bf16 baseline (1.27x).  Verified on the backend at rel(max)
~4.4e-3 (gate 2e-2).
"""

import math
import numpy as np
import ml_dtypes

B, S, D = 2, 2048, 2048
H, KV, HD = 16, 4, 128
G = 4          # tensor-parallel head groups
HG = H // G    # 4 query heads per core
QCOLS = HG * HD  # 512
P = 128
NT = S // P    # 16 sequence tiles
KO = D // P    # 16 contraction chunks
NB = S // 512  # 4 query blocks of 512
NPAIR = KO // 2  # 8 DoubleRow ko pairs

BF16 = ml_dtypes.bfloat16
F8 = ml_dtypes.float8_e4m3

# power-of-2 pre-scales for fp8 (host side); descales fold into
# cos/sin tables (q,k) and the denominator ones-vector (v)
SQK = 512.0   # Wq*HD^-.5 and Wk scale
SV = 64.0     # Wv scale
ONES_VAL = SV  # ones-vector value: recip = 1/(SV*d) so anorm = o_true

_CACHE = {}


def _build_nc():
    import concourse.mybir as mybir
    import concourse.tile as tile
    from concourse import bacc
    from concourse.masks import make_identity
    from contextlib import ExitStack

    dt = mybir.dt
    nc = bacc.Bacc(
        "TRN2",
        target_bir_lowering=False,
        debug=False,
        enable_asserts=False,
        num_devices=8,
    )

    # hT pre-tiled on host: hT*[i, p, ko*128+sc] = split(h.T)[ko*128+p, i*128+sc]
    # so each DMA'd s-tile is one contiguous [128, KO*128] block
    hT_hi = nc.dram_tensor(
        "hT_hi", [NT, P, KO * P], dt.float8e4, kind="ExternalInput"
    ).ap()
    hT_lo = nc.dram_tensor(
        "hT_lo", [NT, P, KO * P], dt.float8e4, kind="ExternalInput"
    ).ap()
    wq_hi = nc.dram_tensor("wq_hi", [D, QCOLS], dt.float8e4, kind="ExternalInput").ap()
    wq_lo = nc.dram_tensor("wq_lo", [D, QCOLS], dt.float8e4, kind="ExternalInput").ap()
    wkv_hi = nc.dram_tensor("wkv_hi", [D, 2 * HD], dt.float8e4, kind="ExternalInput").ap()
    wkv_lo = nc.dram_tensor("wkv_lo", [D, 2 * HD], dt.float8e4, kind="ExternalInput").ap()
    wo = nc.dram_tensor("wo", [QCOLS, D], dt.bfloat16, kind="ExternalInput").ap()
    cosd = nc.dram_tensor("cosd", [S, HD], dt.float32, kind="ExternalInput").ap()
    sind = nc.dram_tensor("sind", [S, HD], dt.float32, kind="ExternalInput").ap()
    y = nc.dram_tensor("y", [S, D], dt.float32, kind="ExternalOutput").ap()

    with tile.TileContext(nc) as tc:
        _emit(tc, nc, mybir, hT_hi, hT_lo, wq_hi, wq_lo, wkv_hi, wkv_lo,
              wo, cosd, sind, y, make_identity)

    nc.compile()
    return nc


def _emit(tc, nc, mybir, hT_hi, hT_lo, wq_hi, wq_lo, wkv_hi, wkv_lo,
          wo, cosd, sind, y, make_identity):
    import os
    from contextlib import ExitStack

    PHASES = os.environ.get("K_PHASES", "123")

    dt = mybir.dt
    bf16 = dt.bfloat16
    f8 = dt.float8e4
    f32 = dt.float32
    Exp = mybir.ActivationFunctionType.Exp
    DR = mybir.MatmulPerfMode.DoubleRow

    with ExitStack() as ctx:
        const = ctx.enter_context(tc.tile_pool(name="const", bufs=1))
        wpool = ctx.enter_context(tc.tile_pool(name="wpool", bufs=1))
        big = ctx.enter_context(tc.tile_pool(name="big", bufs=1))
        hpool = ctx.enter_context(tc.tile_pool(name="hpool", bufs=6))
        work = ctx.enter_context(tc.tile_pool(name="work", bufs=4))
        expp = ctx.enter_context(tc.tile_pool(name="expp", bufs=22))
        # PSUM (8 banks): wide 2x2, a 1x1, c 1x1, y 2x1.  "a" gets away with
        # one buf because its consumers chain anyway (ps_t2 transposes read
        # the anorm derived from ps_oq in the same slot); "y" needs two so
        # an o_proj piece's matmuls overlap the previous piece's copy-out.
        ps_wide = ctx.enter_context(tc.tile_pool(name="ps_wide", bufs=2, space="PSUM"))
        ps_a = ctx.enter_context(tc.tile_pool(name="ps_a", bufs=1, space="PSUM"))
        ps_c = ctx.enter_context(tc.tile_pool(name="ps_c", bufs=1, space="PSUM"))
        ps_y = ctx.enter_context(tc.tile_pool(name="ps_y", bufs=2, space="PSUM"))

        # --- constants ---
        ident = const.tile([P, P], bf16)
        make_identity(nc, ident)
        ident8 = const.tile([P, P], f8)
        make_identity(nc, ident8)
        ones_v = const.tile([P, 1], bf16)
        nc.vector.memset(ones_v, ONES_VAL)
        zero_w = const.tile([P, P], bf16)
        nc.vector.memset(zero_w, 0.0)

        # --- hT prefetch helper (pre-tiled on host; hi/lo halves) ---
        ht_tiles = {}

        def load_ht(i):
            if i not in ht_tiles:
                ht_t = hpool.tile([P, 2, KO, P], f8, tag="ht", name=f"ht{i}")
                nc.sync.dma_start(
                    ht_t[:, 0], hT_hi[i].rearrange("p (ko s) -> p ko s", ko=KO)
                )
                nc.sync.dma_start(
                    ht_t[:, 1], hT_lo[i].rearrange("p (ko s) -> p ko s", ko=KO)
                )
                ht_tiles[i] = ht_t
            return ht_tiles[i]

        # --- weights and tables to SBUF ---
        # startup-critical DMA order: wq_hi + hT(0) feed the first matmul
        # term; lo parts are consumed within the first s-tile.
        wq8 = [wpool.tile([P, KO, QCOLS], f8, name=f"wq8_{t}") for t in range(2)]
        wkv8 = [wpool.tile([P, KO, 2 * HD], f8, name=f"wkv8_{t}") for t in range(2)]
        cos_sb = wpool.tile([P, NT, HD], f32)
        sin_sb = wpool.tile([P, NT, HD], f32)
        wq_r = [wq_hi.rearrange("(ko p) m -> p ko m", p=P),
                wq_lo.rearrange("(ko p) m -> p ko m", p=P)]
        wkv_r = [wkv_hi.rearrange("(ko p) m -> p ko m", p=P),
                 wkv_lo.rearrange("(ko p) m -> p ko m", p=P)]
        cos_r = cosd.rearrange("(i p) c -> p i c", p=P)
        sin_r = sind.rearrange("(i p) c -> p i c", p=P)
        KG = 4  # ko chunks per DMA
        # DMA ladder ordered by consumption time: tile-0's terms consume
        # wq_hi+ht0.hi, then ht0.lo, then wq_lo; hT tiles 1..3 are slotted
        # between weight chunks so tiles 1-3 never wait; cos/sin (consumed
        # by the lagging DVE RoPE chain) trail, with chunks 4+ emitted
        # inside the loop.
        nc.sync.dma_start(wq8[0][:, 0:2], wq_r[0][:, 0:2])
        if "1" in PHASES:
            load_ht(0)
        nc.sync.dma_start(wq8[0][:, 2:4], wq_r[0][:, 2:4])
        nc.sync.dma_start(wkv8[0][:, 0:KG], wkv_r[0][:, 0:KG])
        if "1" in PHASES:
            load_ht(1)
        for kg in range(KG, KO, KG):
            ks = slice(kg, kg + KG)
            nc.sync.dma_start(wq8[0][:, ks], wq_r[0][:, ks])
            nc.sync.dma_start(wkv8[0][:, ks], wkv_r[0][:, ks])
        nc.sync.dma_start(wq8[1][:, 0:KG], wq_r[1][:, 0:KG])
        nc.sync.dma_start(wkv8[1][:, 0:KG], wkv_r[1][:, 0:KG])
        if "1" in PHASES:
            load_ht(2)
        for kg in range(KG, KO, KG):
            ks = slice(kg, kg + KG)
            nc.sync.dma_start(wq8[1][:, ks], wq_r[1][:, ks])
        if "1" in PHASES:
            load_ht(3)
            load_ht(4)
        for kg in range(KG, KO, KG):
            ks = slice(kg, kg + KG)
            nc.sync.dma_start(wkv8[1][:, ks], wkv_r[1][:, ks])
        nc.sync.dma_start(cos_sb[:, 0:KG], cos_r[:, 0:KG])
        nc.sync.dma_start(sin_sb[:, 0:KG], sin_r[:, 0:KG])

        # --- persistent intermediates ---
        # qT and kT fused: [hd, 5, s] with slots 0..3 = q heads, slot 4 = k
        qkT = big.tile([P, HG + 1, S], bf16)
        qT = qkT[:, :HG]                   # [hd, head, s]
        kT = qkT[:, HG]                    # [hd, s]
        v_sb = big.tile([P, NT, HD], bf16)  # [s_inner, s_chunk, hd]
        aoT = big.tile([P, HG, S], bf16)   # attn_out^T  [c_inner, head, s]

        # --- attention helpers (shared by the interleaved head-0 pipeline
        # --- and the phase-2 block loop) ---
        def emit_scores(h, qs, j):
            ps_s2 = ps_wide.tile([P, 1024], f32, tag="wide", name="ps_s2")
            for r in range(2):
                c = 2 * j + r
                nc.tensor.matmul(
                    ps_s2[:, r * 512 : (r + 1) * 512],
                    kT[:, c * P : (c + 1) * P],
                    qT[:, h, qs],
                    start=True,
                    stop=True,
                )
            expT = expp.tile([P, 1024], bf16, tag="exp", name="expT")
            nc.scalar.activation(expT, ps_s2, Exp)
            return expT

        def emit_pv_all(ps_oq, ps_sum, j, expT):
            # flipped PV: exp slice stationary, v moving; the denominator
            # rides along as a 1-column matmul on the same stationary
            # operand (~free on the PE).  All four q-subchunks are emitted
            # per pair so each pair costs ~1.28us of PE work vs ~1.04us of
            # ScalarE exp — the exp stream latency is self-covered.  The
            # PSUM groups accumulate with start=False onto a gpsimd-zeroed
            # bank (per-instruction overwrite bits can't express four
            # interleaved column groups in one bank).
            for r in range(2):
                c = 2 * j + r
                for qq in range(HG):
                    sl = expT[:, r * 512 + qq * P : r * 512 + (qq + 1) * P]
                    nc.tensor.matmul(
                        ps_oq[:, qq], sl, v_sb[:, c],
                        start=False, stop=False,
                        skip_group_check=True,
                    )
                    nc.tensor.matmul(
                        ps_sum[:, qq : qq + 1], sl, ones_v,
                        start=False, stop=False,
                        skip_group_check=True,
                    )

        # head 0 of block 0's SCORE pairs run interleaved with phase 1:
        # pair j only needs kT tiles <= 2j+1 and qT tiles 0..3, and with
        # ps_q banished to the "y" slots the wide PSUM slots are free —
        # so its exps stream where ScalarE is otherwise idle, and block 0
        # starts with 8 resident exp tiles (pure-PE PV work).
        carry = {}

        def emit_h0_pair(j):
            if "2" not in PHASES:
                return
            pre = carry.setdefault((0, 0), [])
            assert len(pre) == j
            pre.append(emit_scores(0, slice(0, 512), j))

        # pair j is emitted after phase-1 tile max(5, 2j+1)
        H0_SCHED = {5: [0, 1], 6: [2], 7: [3], 9: [4], 11: [5], 13: [6],
                    15: [7]}

        # ---------------- Phase 1: QKV projections + RoPE + transposes ------
        late_qrot = []
        for i in range(NT if "1" in PHASES else 0):
            ht_t = load_ht(i)
            if i + 3 < NT:
                load_ht(i + 3)
            if i in (3, 5, 7):
                ts_ = slice(2 * (i - 1), 2 * (i - 1) + KG)
                nc.sync.dma_start(cos_sb[:, ts_], cos_r[:, ts_])
                nc.sync.dma_start(sin_sb[:, ts_], sin_r[:, ts_])

            # ps_q lives in a "y" slot during phase 1 (o_proj is idle), so
            # the wide slots stay free for block-0 head-0's score pairs;
            # tiles 0-1 borrow the wide slots so the slow lo-weight DMAs
            # never gate the early psum rotation
            if i < 2:
                ps_q = ps_wide.tile([P, QCOLS], f32, tag="wide", name="ps_q")
            else:
                ps_q = ps_y.tile([P, QCOLS], f32, tag="y", name="ps_q")
            ps_kv = ps_c.tile([P, 2 * HD], f32, tag="c", name="ps_kv")
            # 3-term fp8 DoubleRow: hh*wh (all pairs), hl*wh, hh*wl — the
            # wl term last so the lo-weight DMAs are off the startup path
            terms = [(0, 0), (1, 0), (0, 1)]
            n_mm = len(terms) * NPAIR
            mm = 0
            # q-major then kv-major: matches the DMA arrival ladder so the
            # early tiles' ready work stays dense
            for (ha, wb) in terms:
                for j in range(NPAIR):
                    js = slice(2 * j, 2 * j + 2)
                    nc.tensor.matmul(
                        ps_q, ht_t[:, ha, js], wq8[wb][:, js],
                        start=(mm == 0), stop=(mm == n_mm - 1), perf_mode=DR,
                    )
                    mm += 1
            mm = 0
            for (ha, wb) in terms:
                for j in range(NPAIR):
                    js = slice(2 * j, 2 * j + 2)
                    nc.tensor.matmul(
                        ps_kv, ht_t[:, ha, js], wkv8[wb][:, js],
                        start=(mm == 0), stop=(mm == n_mm - 1), perf_mode=DR,
                    )
                    mm += 1

            # v: straight cast copy into [s, hd] layout; route the last
            # tiles' copies to DVE so ACT is free when attention starts
            cp = nc.vector if i >= NT - 3 else nc.scalar
            if cp is nc.vector:
                nc.vector.tensor_copy(v_sb[:, i], ps_kv[:, HD:])
            else:
                nc.scalar.copy(v_sb[:, i], ps_kv[:, HD:])

            # q and k side by side in one [P, 5, HD] fp32 tile for fused RoPE
            qk_f = work.tile([P, HG + 1, HD], f32, tag="qkf")
            if cp is nc.vector:
                nc.vector.tensor_copy(
                    qk_f[:, :HG], ps_q.rearrange("p (h c) -> p h c", h=HG)
                )
                nc.vector.tensor_copy(qk_f[:, HG], ps_kv[:, :HD])
            else:
                nc.scalar.copy(
                    qk_f[:, :HG], ps_q.rearrange("p (h c) -> p h c", h=HG)
                )
                nc.scalar.copy(qk_f[:, HG], ps_kv[:, :HD])

            HF = HD // 2

            def do_rope(src, lo_h, n_h, i=i):
                # returns bf16 RoPE(src[:, lo_h:lo_h+n_h]) as [P, n_h, HD];
                # cos/sin tables carry the 1/SQK descale (host-folded)
                cos_t = cos_sb[:, i]
                sin_t = sin_sb[:, i]
                cos_lo = cos_t[:, None, :HF].to_broadcast((P, n_h, HF))
                cos_hi = cos_t[:, None, HF:].to_broadcast((P, n_h, HF))
                sin_lo = sin_t[:, None, :HF].to_broadcast((P, n_h, HF))
                sin_hi = sin_t[:, None, HF:].to_broadcast((P, n_h, HF))
                s = src[:, lo_h : lo_h + n_h]
                s_lo = s[:, :, :HF]
                s_hi = s[:, :, HF:]
                rot = work.tile(
                    [P, HG + 1, HD], bf16, tag="qkrot", name="rot"
                )[:, :n_h]
                t1 = work.tile([P, HG + 1, HF], f32, tag="rt1", name="t1")[:, :n_h]
                t2 = work.tile([P, HG + 1, HF], f32, tag="rt2", name="t2")[:, :n_h]
                nc.vector.tensor_mul(t1, s_lo, cos_lo)
                nc.vector.tensor_mul(t2, s_hi, sin_lo)
                nc.vector.tensor_sub(rot[:, :, :HF], t1, t2)
                t3 = work.tile([P, HG + 1, HF], f32, tag="rt1", name="t3")[:, :n_h]
                t4 = work.tile([P, HG + 1, HF], f32, tag="rt2", name="t4")[:, :n_h]
                nc.vector.tensor_mul(t3, s_hi, cos_hi)
                nc.vector.tensor_mul(t4, s_lo, sin_hi)
                nc.vector.tensor_add(rot[:, :, HF:], t3, t4)
                return rot

            if i < 4 * (NB - 1):
                # fused RoPE over q heads + k, then all 5 transposes
                qk_rot = do_rope(qk_f, 0, HG + 1)
                ps_tk = ps_a.tile([P, P], bf16, tag="a", name="ps_tk")
                nc.tensor.transpose(ps_tk, qk_rot[:, HG], ident)
                nc.vector.tensor_copy(kT[:, i * P : (i + 1) * P], ps_tk)
                ps_t = ps_a.tile([P, HG * P], bf16, tag="a", name="ps_t")
                for h in range(HG):
                    nc.tensor.transpose(
                        ps_t[:, h * P : (h + 1) * P], qk_rot[:, h], ident
                    )
                nc.vector.tensor_copy(
                    qT[:, :, i * P : (i + 1) * P],
                    ps_t.rearrange("p (h s) -> p h s", h=HG),
                )
            else:
                # last block: narrow k-only RoPE first (kT gates ALL of
                # phase 2); q RoPE + transposes deferred past the boundary
                k_rot = do_rope(qk_f, HG, 1)
                ps_tk = ps_a.tile([P, P], bf16, tag="a", name="ps_tk")
                nc.tensor.transpose(ps_tk, k_rot[:, 0], ident)
                nc.vector.tensor_copy(kT[:, i * P : (i + 1) * P], ps_tk)
                late_qrot.append((i, qk_f, do_rope))

            for _j in H0_SCHED.get(i, []):
                emit_h0_pair(_j)

        h0ps = {}
        if "2" in PHASES and "1" in PHASES:
            h0ps["oq"] = ps_y.tile([P, HG, P], f32, tag="y", name="ps_oq0")
            h0ps["sum"] = ps_c.tile([P, HG], f32, tag="c", name="ps_sum0")
            nc.vector.memset(h0ps["oq"], 0.0)
            nc.vector.memset(h0ps["sum"], 0.0)

        # wo is only needed for o_proj: load it while phase 2 runs
        wo_sb = wpool.tile([P, HG, D], bf16)
        nc.sync.dma_start(wo_sb, wo.rearrange("(ch p) n -> p ch n", p=P))

        # ------- Phase 2 (attention) with phase-3 o_proj pieces interleaved
        y_r = y.rearrange("(i p) n -> p i n", p=P)
        ysel = [0]

        def emit_oproj_piece(i, piece, pool=None, split=False):
            # one [128,512] slice of y[q_tile i] (own 2-buf PSUM tag so a
            # piece's matmuls overlap the previous piece's copy-out; the
            # tail also rotates through the then-idle wide slots).  The very
            # last piece is copied/DMA'd in two halves so the closing DMA
            # is small and starts early.
            ns = slice(piece * 512, (piece + 1) * 512)
            if pool is None:
                psy = ps_y.tile([P, 512], f32, tag="y", name="psy")
            else:
                psy = ps_wide.tile([P, 512], f32, tag="wide", name="psyw")
            for ch in range(HG):
                nc.tensor.matmul(
                    psy, aoT[:, ch, i * P : (i + 1) * P], wo_sb[:, ch, ns],
                    start=(ch == 0), stop=(ch == HG - 1),
                )
            y_sb = work.tile([P, 512], f32, tag="ysb", bufs=6)
            if split:
                nc.vector.tensor_copy(y_sb[:, :256], psy[:, :256])
                nc.sync.dma_start(
                    y_r[:, i, piece * 512 : piece * 512 + 256], y_sb[:, :256]
                )
                nc.scalar.copy(y_sb[:, 256:], psy[:, 256:])
                nc.sync.dma_start(
                    y_r[:, i, piece * 512 + 256 : (piece + 1) * 512],
                    y_sb[:, 256:],
                )
                return
            if ysel[0] % 2 == 0:
                nc.vector.tensor_copy(y_sb, psy)
            else:
                nc.scalar.copy(y_sb, psy)
            ysel[0] += 1
            nc.sync.dma_start(y_r[:, i, ns], y_sb)

        def emit_late_qrot(idx):
            # deferred q RoPE + transposes for the last attention block;
            # spread one tile per block-1 head so the single "a" PSUM slot's
            # rotation never stalls the ps_oq pipeline
            i_l, qk_f_l, rope_fn = late_qrot[idx]
            q_rot_l = rope_fn(qk_f_l, 0, HG)
            ps_t = ps_a.tile([P, HG * P], bf16, tag="a", name="ps_tl")
            for h in range(HG):
                nc.tensor.transpose(
                    ps_t[:, h * P : (h + 1) * P], q_rot_l[:, h], ident
                )
            nc.vector.tensor_copy(
                qT[:, :, i_l * P : (i_l + 1) * P],
                ps_t.rearrange("p (h s) -> p h s", h=HG),
            )

        NPRE = 8  # next-head score pairs pre-emitted during current head
        bh_list = [(b, h) for b in range(NB if "2" in PHASES else 0)
                   for h in range(HG)]
        for bi, (b, h) in enumerate(bh_list):
            qs = slice(b * 512, (b + 1) * 512)
            # previous block's o_proj pieces are spread between the PV
            # subchunk loops so the PE always has fill-in work while
            # exps stream / the DVE normalize chain runs
            op_tile = 4 * (b - 1) + h if ("3" in PHASES and b >= 1) else None
            nxt = bh_list[bi + 1] if bi + 1 < len(bh_list) else None

            if b == 0 and h == 0 and h0ps:
                ps_oq, ps_sum = h0ps["oq"], h0ps["sum"]
            else:
                ps_oq = ps_a.tile([P, HG, P], f32, tag="a", name="ps_oq")
                ps_sum = ps_c.tile([P, HG], f32, tag="c", name="ps_sum")
                # zero via PE matmul (start=True writes 0*qkT): keeps the
                # inter-head chain on the PE instead of a serial DVE memset
                nc.tensor.matmul(ps_oq, zero_w,
                                 qkT[:, 0, : HG * P].rearrange(
                                     "p (g c) -> p g c", g=HG),
                                 start=True, stop=True, skip_group_check=True)
                nc.tensor.matmul(ps_sum, zero_w, qkT[:, 0, :HG],
                                 start=True, stop=True, skip_group_check=True)
            exps = carry.pop((b, h), [])

            def pre_emit_one():
                # pre-emit one of the next head's score pairs so ScalarE
                # keeps streaming exps across the head boundary
                if nxt is None:
                    return
                nb_, nh_ = nxt
                pre = carry.setdefault(nxt, [])
                if len(pre) < NPRE:
                    pre.append(emit_scores(
                        nh_, slice(nb_ * 512, (nb_ + 1) * 512), len(pre)))

            n_carried = len(exps)
            for j in range(NT // 2):
                if j >= n_carried:
                    exps.append(emit_scores(h, qs, j))
                if j >= 1:
                    emit_pv_all(ps_oq, ps_sum, j - 1, exps[j - 1])
                    pre_emit_one()
                if j == 4 and op_tile is not None:
                    emit_oproj_piece(op_tile, 0)
            emit_pv_all(ps_oq, ps_sum, NT // 2 - 1, exps[-1])
            if op_tile is not None:
                emit_oproj_piece(op_tile, 1)
                emit_oproj_piece(op_tile, 2)
            for _ in range(NPRE):
                pre_emit_one()

            rec = work.tile([P, HG], f32, tag="rec")
            nc.vector.reciprocal(rec, ps_sum)
            anorm = work.tile([P, HG, P], bf16, tag="anorm")
            nc.vector.tensor_mul(
                anorm, ps_oq, rec[:, :, None].to_broadcast((P, HG, P))
            )
            ps_t2 = ps_a.tile([P, HG * P], bf16, tag="a", name="ps_t2")
            for t in range(HG):
                nc.tensor.transpose(
                    ps_t2[:, t * P : (t + 1) * P], anorm[:, t], ident
                )
            nc.vector.tensor_copy(
                aoT[:, h, qs],
                ps_t2,
            )

            if op_tile is not None:
                emit_oproj_piece(op_tile, 3)
            if b == 0 and late_qrot:
                emit_late_qrot(h)

        # ---------------- Phase 3 tail: last block's o_proj ----------------
        if "3" in PHASES and "2" in PHASES:
            np_ = 0
            for i in range(4 * (NB - 1), 4 * NB):
                for piece in range(4):
                    # 4-deep psum rotation: y,y,wide,wide
                    emit_oproj_piece(i, piece, pool="wide" if np_ % 4 >= 2 else None)
                    np_ += 1


def get_nc():
    if "nc" not in _CACHE:
        _CACHE["nc"] = _build_nc()
    return _CACHE["nc"]


def _split8(x):
    """hi/lo fp8e4m3 split of x (float32 in)."""
    hi = x.astype(F8)
    lo = (x - hi.astype(np.float32)).astype(F8)
    return hi, lo


def make_in_maps(inputs):
    """Shard full inputs into 8 per-core input maps."""
    h = np.asarray(inputs["hidden_states"], dtype=np.float32)
    cos = np.asarray(inputs["cos"], dtype=np.float32).reshape(S, HD) / SQK
    sin = np.asarray(inputs["sin"], dtype=np.float32).reshape(S, HD) / SQK
    # fold the 1/sqrt(HD) softmax scale + fp8 pre-scale into Wq
    Wq = np.asarray(inputs["Wq"], dtype=np.float32) * (HD ** -0.5) * SQK
    Wk = np.asarray(inputs["Wk"], dtype=np.float32) * SQK
    Wv = np.asarray(inputs["Wv"], dtype=np.float32) * SV
    Wo = np.asarray(inputs["Wo"], dtype=np.float32)

    # hT[i, p, ko*128+sc] = h[b].T[ko*128+p, i*128+sc]  (see dram decl)
    def tile_hT(hb):
        return np.ascontiguousarray(
            hb.T.reshape(KO, P, NT, P).transpose(2, 1, 0, 3).reshape(NT, P, KO * P)
        )

    hT8 = [_split8(tile_hT(h[b])) for b in range(B)]
    wq8 = [_split8(Wq[:, g * QCOLS : (g + 1) * QCOLS]) for g in range(G)]
    wkv8 = [
        _split8(
            np.concatenate(
                [Wk[:, g * HD : (g + 1) * HD], Wv[:, g * HD : (g + 1) * HD]],
                axis=1,
            )
        )
        for g in range(G)
    ]
    wo_s = [
        np.ascontiguousarray(Wo[g * QCOLS : (g + 1) * QCOLS, :]).astype(BF16)
        for g in range(G)
    ]

    in_maps = []
    for core in range(8):
        b, g = divmod(core, G)
        in_maps.append(
            {
                "hT_hi": hT8[b][0],
                "hT_lo": hT8[b][1],
                "wq_hi": wq8[g][0],
                "wq_lo": wq8[g][1],
                "wkv_hi": wkv8[g][0],
                "wkv_lo": wkv8[g][1],
                "wo": wo_s[g],
                "cosd": cos,
                "sind": sin,
            }
        )
    return in_maps


def kernel(**inputs) -> np.ndarray:
    from concourse import bass_utils

    nc = get_nc()
    in_maps = make_in_maps(inputs)
    res = bass_utils.run_bass_kernel_spmd(nc, in_maps, core_ids=list(range(8)))
    out = np.zeros((B, S, D), dtype=np.float32)
    for core in range(8):
        b = core // G
        out[b] += np.asarray(res.results[core]["y"], dtype=np.float32)
    return out
